# revision 1
# baseline (speedup 1.0000x reference)
"""Bidirectional LSTM (L=512, B=64, E=512, H=512 per dir) on 8 NeuronCores.

Strategy (SPMD, zero cross-core communication):
  - Batch-parallel over B: core c owns samples [8c, 8c+8), both directions.
  - Phase 1: embedding gather (indirect DMA) -> X; X.T via PE transposes;
    g_pre = X @ Wih.T + (b_ih + b_hh) with big matmuls; stored to a DRAM
    scratch in source-time order for both directions.
  - Phase 2: 512 fully-unrolled recurrence steps. Per step, gates
    g = g_pre[t] + h @ Whh.T accumulate in PSUM: h-part via 4 K-chunk
    matmuls, g_pre injected through the PE with an eye(16) stationary
    (DMA cannot touch PSUM). The four (direction, h-half) units map to the
    four 32-column groups of the PE array / PSUM partition blocks
    (base partitions 0/32/64/96, 8 rows each) so their matmuls execute
    concurrently; emission is wave-interleaved across groups.
  - Gate columns are host-permuted to [i|f|o|g] per 256-wide h-half so one
    sigmoid op covers i,f,o and one tanh covers g.
  - Padding mask folded into the sigmoid bias (per-partition bias AP):
    sigma(x - 1e9*(1-m)) == 0 at padded steps => c_t = h_t = 0 exactly as
    the reference's post-step h*m, c*m masking (mask is monotone).
  - h.T for the next step via PE transposes (cols land at the partition
    block offsets, directly usable as the next matmul's stationary).
"""

import os
import sys

sys.path.insert(0, "/opt/trn_rl_repo")

import numpy as np

L, B, E, V = 512, 64, 512, 32000
H = 512           # hidden per direction
NB = 8            # batch per core
NCORES = 8
HH = 256          # h per half
GW = 1024         # gate cols per half (4 gates x 256)

_BUILT = {}


def _split_sync_waits(nc, max_waits=1):
    """This container's walrus rejects >1 sync-wait per instruction
    (CoreV3GenImpl setupSyncWait). Split extras onto preceding same-engine
    NoOps."""
    import concourse.mybir as mybir

    ctr = 0
    for fn in nc.m.functions:
        for blk in fn.blocks:
            out = []
            changed = False
            for inst in blk.instructions:
                si = inst.sync_info
                if si is not None and si.on_wait and len(si.on_wait) > max_waits:
                    waits = list(si.on_wait)
                    extra, keep = waits[:-max_waits], waits[-max_waits:]
                    for i in range(0, len(extra), max_waits):
                        ctr += 1
                        nop = mybir.InstNoOp(
                            name=f"bass_waitsplit_{ctr}", ins=[], outs=[])
                        nop.engine = inst.engine
                        nop.sync_info = mybir.SyncInfo(
                            on_wait=extra[i:i + max_waits], on_update=[])
                        out.append(nop)
                    si.on_wait = keep
                    changed = True
                out.append(inst)
            if changed:
                blk.instructions[:] = out


def _gate_perm():
    """New gate-column order (length 4H): per half q in {0,1}:
    [i[256q:256q+256], f[...], o[...], g[...]] referencing original rows
    i=0:512, f=512:1024, g=1024:1536, o=1536:2048."""
    p = []
    for q in range(2):
        s = 256 * q
        p += list(range(s, s + 256))            # i
        p += list(range(512 + s, 512 + s + 256))   # f
        p += list(range(1536 + s, 1536 + s + 256))  # o
        p += list(range(1024 + s, 1024 + s + 256))  # g
    return np.array(p, dtype=np.int64)


def _build(nsteps=L, ntiles=L * NB // 128):
    key = (nsteps, ntiles)
    if key in _BUILT:
        return _BUILT[key]
    import concourse.bass as bass
    import concourse.mybir as mybir
    import concourse.tile as tile
    from concourse.masks import make_identity

    f32 = mybir.dt.float32
    nrows = ntiles * 128

    nc = bass.Bass()
    emb = nc.dram_tensor("emb", [V, E], f32, kind="ExternalInput")
    toks = nc.dram_tensor("toks", [128, ntiles], mybir.dt.int32,
                          kind="ExternalInput")
    tokmask = nc.dram_tensor("tokmask", [128, ntiles], f32,
                             kind="ExternalInput")
    sigbias = nc.dram_tensor("sigbias", [128, nsteps], f32, kind="ExternalInput")
    wihT_d = nc.dram_tensor("wihT", [2, 4, 128, 2048], f32, kind="ExternalInput")
    whhT_d = nc.dram_tensor("whhT", [2, 4, 128, 2048], f32, kind="ExternalInput")
    gbias_d = nc.dram_tensor("gbias", [2, 2048], f32, kind="ExternalInput")
    out_d = nc.dram_tensor("out", [nsteps, NB, 2 * H], f32, kind="ExternalOutput")

    with tile.TileContext(nc) as tc:
        with (
            tc.tile_pool(name="persist", bufs=1) as pp,
            tc.tile_pool(name="dram", bufs=1, space="DRAM") as dp,
        ):
            # ---- persistent SBUF ----
            wihT = pp.tile([128, 2, 4, 2048], f32)
            whhT = pp.tile([128, 2, 4, 2048], f32)
            for d in range(2):
                for k in range(4):
                    nc.sync.dma_start(wihT[:, d, k, :], wihT_d[d, k])
                    nc.sync.dma_start(whhT[:, d, k, :], whhT_d[d, k])
            gbias = pp.tile([1, 2, 2048], f32)
            nc.sync.dma_start(gbias[:, 0, :], gbias_d[0:1, :])
            nc.sync.dma_start(gbias[:, 1, :], gbias_d[1:2, :])
            sb = pp.tile([128, nsteps], f32)
            nc.sync.dma_start(sb[:], sigbias[:])
            ident = pp.tile([128, 128], f32)
            make_identity(nc, ident[:])
            ones1 = pp.tile([1, 128], f32)
            nc.vector.memset(ones1[:], 1.0)
            toks_t = pp.tile([128, ntiles], mybir.dt.int32)
            nc.sync.dma_start(toks_t[:], toks[:])
            tmask_t = pp.tile([128, ntiles], f32)
            nc.sync.dma_start(tmask_t[:], tokmask[:])

            gpre = dp.tile([nsteps, 16, 2048], f32)

            # ================= Phase 1: g_pre =================
            with (
                tc.tile_pool(name="p1", bufs=3) as p1,
                tc.tile_pool(name="p1ps", bufs=2, space="PSUM") as p1ps,
                tc.tile_pool(name="p1tr", bufs=2, space="PSUM") as p1tr,
            ):
                for r in range(ntiles):
                    xt = p1.tile([128, E], f32)
                    nc.gpsimd.indirect_dma_start(
                        out=xt[:], out_offset=None, in_=emb[:],
                        in_offset=bass.IndirectOffsetOnAxis(
                            ap=toks_t[:, r:r + 1], axis=0))
                    nc.vector.tensor_scalar_mul(xt[:], xt[:], tmask_t[:, r:r + 1])
                    xT = p1.tile([128, 4, 128], f32)
                    for k in range(4):
                        trp = p1tr.tile([128, 128], f32, space="PSUM")
                        nc.tensor.transpose(
                            out=trp[:], in_=xt[:, 128 * k:128 * (k + 1)],
                            identity=ident[:])
                        if k % 2 == 0:
                            nc.vector.tensor_copy(xT[:, k, :], trp[:])
                        else:
                            nc.scalar.copy(xT[:, k, :], trp[:])
                    for d in range(2):
                        for nch in range(4):
                            n0 = 512 * nch
                            gps = p1ps.tile([128, 512], f32, space="PSUM")
                            for k in range(4):
                                nc.tensor.matmul(
                                    gps[:], xT[:, k, :],
                                    wihT[:, d, k, n0:n0 + 512],
                                    start=(k == 0), stop=False)
                            nc.tensor.matmul(
                                gps[:], ones1[:], gbias[:, d, n0:n0 + 512],
                                start=False, stop=True)
                            gsb = p1.tile([128, 512], f32)
                            if nch % 2 == 0:
                                nc.vector.tensor_copy(gsb[:], gps[:])
                            else:
                                nc.scalar.copy(gsb[:], gps[:])
                            # rows of this tile are (l = 16r + i, b); write to
                            # gpre[l, 8d + b, n0:n0+512]
                            nc.sync.dma_start(
                                gpre[16 * r:16 * (r + 1),
                                     8 * d:8 * d + 8, n0:n0 + 512],
                                gsb[:])

            # ================= Phase 2: recurrence =================
            with (
                tc.tile_pool(name="p2", bufs=2) as p2,
                tc.tile_pool(name="p2g", bufs=4) as p2g,
                tc.tile_pool(name="p2ps", bufs=2, space="PSUM") as p2ps,
                tc.tile_pool(name="p2tr", bufs=4, space="PSUM") as p2tr,
            ):
                hT_prev = None
                c_prev = None
                for t in range(nsteps):
                    gp = p2g.tile([16, 2048], f32)
                    nc.sync.dma_start(gp[0:8, :], gpre[t, 0:8, :])
                    nc.sync.dma_start(
                        gp[8:16, :], gpre[nsteps - 1 - t, 8:16, :])

                    gps = p2ps.tile([128, 1024], f32, space="PSUM")
                    # wave-interleaved matmuls across the 4 groups
                    # group g: direction d = g >> 1, half q = g & 1,
                    # psum partitions [32g, 32g+8)
                    # M=32 everywhere (cols 8:32 of each group compute garbage
                    # from uninitialized lanes; block-diagonal so it never
                    # touches the real 8 rows) so the whole PSUM tile is
                    # written and downstream full-width reads are clean.
                    for nb_ in range(2):
                        pcol = 512 * nb_
                        if hT_prev is not None:
                            for k in range(4):
                                for g in range(4):
                                    d, q = g >> 1, g & 1
                                    n0 = GW * q + pcol
                                    lcol = 32 * (k // 2) + 64 * d
                                    # per-group start: clears the 2KB zero
                                    # region within this group's partitions
                                    # only. skip_group_check silences the
                                    # sim's partition-blind group tracker.
                                    nc.tensor.matmul(
                                        gps[32 * g:32 * g + 32, pcol:pcol + 512],
                                        hT_prev[:, k, lcol:lcol + 32],
                                        whhT[:, d, k, n0:n0 + 512],
                                        start=(k == 0), stop=False,
                                        tile_position=(0, 32 * g),
                                        skip_group_check=True)
                        for g in range(4):
                            d, q = g >> 1, g & 1
                            n0 = GW * q + pcol
                            # eye cols 8d:8d+32: row j<8 picks gpre row 8d+j,
                            # rows 8..32 hit eye rows >=16 -> zero
                            nc.tensor.matmul(
                                gps[32 * g:32 * g + 32, pcol:pcol + 512],
                                ident[0:16, 8 * d:8 * d + 32],
                                gp[:, n0:n0 + 512],
                                start=(hT_prev is None), stop=True,
                                tile_position=(0, 32 * g),
                                skip_group_check=True)

                    sig = p2.tile([128, 768], f32)
                    nc.scalar.activation(
                        sig[:], gps[:, 0:768],
                        mybir.ActivationFunctionType.Sigmoid,
                        bias=sb[:, t:t + 1], scale=1.0)
                    tg = p2.tile([128, 256], f32)
                    nc.scalar.activation(
                        tg[:], gps[:, 768:1024],
                        mybir.ActivationFunctionType.Tanh)

                    c_new = p2.tile([128, 256], f32, tag="c_state")
                    if c_prev is None:
                        nc.vector.tensor_mul(c_new[:], sig[:, 0:256], tg[:])
                    else:
                        t1 = p2.tile([128, 256], f32)
                        nc.vector.tensor_mul(t1[:], sig[:, 0:256], tg[:])
                        t2 = p2.tile([128, 256], f32)
                        nc.vector.tensor_mul(t2[:], sig[:, 256:512], c_prev[:])
                        nc.vector.tensor_add(c_new[:], t1[:], t2[:])
                    tc_ = p2.tile([128, 256], f32)
                    nc.scalar.activation(
                        tc_[:], c_new[:], mybir.ActivationFunctionType.Tanh)
                    h = p2.tile([128, 256], f32)
                    nc.vector.tensor_mul(h[:], sig[:, 512:768], tc_[:])

                    hT = p2.tile([128, 4, 128], f32, tag="hT_state")
                    for k in range(4):
                        off = 128 * (k % 2)
                        trp = p2tr.tile([128, 128], f32, space="PSUM")
                        nc.tensor.transpose(
                            out=trp[:], in_=h[:, off:off + 128],
                            identity=ident[:])
                        if k % 2 == 0:
                            nc.vector.tensor_copy(hT[:, k, :], trp[:])
                        else:
                            nc.scalar.copy(hT[:, k, :], trp[:])

                    # output: fwd -> out[t, :, 0:512]; bwd -> out[L-1-t, :, 512:1024]
                    nc.sync.dma_start(out_d[t, :, 0:256], h[0:8, :])
                    nc.sync.dma_start(out_d[t, :, 256:512], h[32:40, :])
                    nc.sync.dma_start(
                        out_d[nsteps - 1 - t, :, 512:768], h[64:72, :])
                    nc.sync.dma_start(
                        out_d[nsteps - 1 - t, :, 768:1024], h[96:104, :])

                    hT_prev = hT
                    c_prev = c_new

    _BUILT[key] = nc
    return nc


def _ensure_split(nc):
    if not getattr(nc, "_waitsplit_done", False):
        _split_sync_waits(nc)
        nc._waitsplit_done = True


def _prep_core_inputs(c, tokens, mask, emb_table, wihT, whhT, gbias, sigbias_all,
                      nsteps, ntiles):
    s = slice(NB * c, NB * (c + 1))
    # row r*128+p of the (l, b) flattening, laid out [partition, tile]
    toks_c = np.clip(tokens[:nsteps, s], 0, V - 1).astype(np.int32)
    toks_c = toks_c.reshape(ntiles, 128).T
    tmask_c = mask[:nsteps, s].astype(np.float32).reshape(ntiles, 128).T
    return {
        "emb": emb_table,
        "toks": np.ascontiguousarray(toks_c),
        "tokmask": np.ascontiguousarray(tmask_c),
        "sigbias": np.ascontiguousarray(sigbias_all[c]),
        "wihT": wihT,
        "whhT": whhT,
        "gbias": gbias,
    }


def kernel(tokens, mask, emb_table, W_ih_f, W_hh_f, b_ih_f, b_hh_f,
           W_ih_b, W_hh_b, b_ih_b, b_hh_b, _nsteps=L, _trace=False):
    from concourse.bass_utils import run_bass_kernel_spmd

    tokens = np.asarray(tokens)
    mask = np.asarray(mask, dtype=np.float32)
    emb_table = np.ascontiguousarray(np.asarray(emb_table, dtype=np.float32))

    perm = _gate_perm()
    wihT = np.stack([
        np.asarray(W_ih_f, np.float32)[perm].T.reshape(4, 128, 2048),
        np.asarray(W_ih_b, np.float32)[perm].T.reshape(4, 128, 2048),
    ]).copy()
    whhT = np.stack([
        np.asarray(W_hh_f, np.float32)[perm].T.reshape(4, 128, 2048),
        np.asarray(W_hh_b, np.float32)[perm].T.reshape(4, 128, 2048),
    ]).copy()
    gbias = np.stack([
        (np.asarray(b_ih_f, np.float32) + np.asarray(b_hh_f, np.float32))[perm],
        (np.asarray(b_ih_b, np.float32) + np.asarray(b_hh_b, np.float32))[perm],
    ]).copy()

    nsteps = _nsteps
    ntiles = nsteps * NB // 128

    # sigbias[core][p, t]: fwd blocks (p in [0,8) u [32,40)): -1e9*(1-mask[t, b]);
    # bwd blocks (p in [64,72) u [96,104)): -1e9*(1-mask[L-1-t, b])
    sigbias_all = np.zeros((NCORES, 128, nsteps), np.float32)
    for c in range(NCORES):
        mk = mask[:nsteps, NB * c:NB * (c + 1)]          # [T, 8]
        fwd = -1e9 * (1.0 - mk.T)                        # [8, T]
        bwd = -1e9 * (1.0 - mk[::-1].T)
        for base in (0, 32):
            sigbias_all[c, base:base + 8] = fwd
        for base in (64, 96):
            sigbias_all[c, base:base + 8] = bwd

    nc = _build(nsteps, ntiles)
    _ensure_split(nc)
    in_maps = [
        _prep_core_inputs(c, tokens, mask, emb_table, wihT, whhT, gbias,
                          sigbias_all, nsteps, ntiles)
        for c in range(NCORES)
    ]
    res = run_bass_kernel_spmd(nc, in_maps, core_ids=list(range(NCORES)),
                               trace=_trace)
    out = np.empty((nsteps, B, 2 * H), np.float32)
    for c in range(NCORES):
        out[:, NB * c:NB * (c + 1), :] = res.results[c]["out"]
    kernel._last_results = res
    return out



# revision 33
# speedup vs baseline: 27.8404x; 27.8404x over previous
"""Bidirectional LSTM (L=512, B=64, E=512, H=512 per dir) on 8 NeuronCores.

Strategy (SPMD, zero cross-core communication, batch-parallel: core c owns
samples [8c, 8c+8), both directions):

Transposed recurrence: gates live on PSUM *partitions* (128 gate rows per
(ttype, qg) chunk), batch+hidden-chunk on the free dim.  Cost model charges
matmuls by output free size only, so out free = 8 (batch) makes each of the
64 h-matmuls per direction-step ~3 ns instead of streaming 512x4-cycle fp32
columns like the row-major formulation.

  - All matmul operands fp16 (PSUM accumulates fp32 exactly).
  - tanh folded to sigmoid: tanh(x) = 2*sigmoid(2x)-1, with the 2x folded
    into the g-gate rows of Wih/Whh/bias, so one sigmoid covers all 128
    gate columns per step; the affine 2u-1 runs on DVE/GPSIMD.
  - Gate layout per step column block: c = ttype*32 + b*4 + q with ttype
    order (i, f, o, g); hidden index = q*128 + p.  G PSUM tile [128,4,4,8,4]
    = one full 2KB bank = 4 steps; opened by a K=16 bias matmul
    (start=True zeroes the bank), then mask (K=1), x-part (free 32, batched
    over 4 steps), then per-step h-matmuls; stop on the last h-matmul.
  - Padding: x rows zeroed via embedding mask; i/o gate pre-activations get
    -60000 at padded (t,b) via a rank-1 matmul from a per-4-step-group mask
    table, so sigmoid(i)=sigmoid(o)=0 => h=0 and c propagates exactly like
    the reference's h*m, c*m masking (mask is monotone per direction).
  - fwd and bwd are independent recurrences interleaved step-by-step: two
    chains hide each other's serial activation->elementwise latency.
  - h written per step in fp16 to a history tile; every 8 steps one DMA
    flushes to a DRAM scratch laid out [d, p, t, (b,q)].  The epilogue
    PE-transposes the scratch into [t, b, hidden] fp32 output (fp16
    precision is ample for rel tol 2e-2).
"""

import sys

sys.path.insert(0, "/opt/trn_rl_repo")

import numpy as np

L, B, E, V = 512, 64, 512, 32000
H = 512
NB = 8
NCORES = 8
NT = L * NB // 128      # 32 gather tiles of 128 (l,b) rows

_BUILT = {}

# ttype order (i, f, o, g) -> original row block offsets (i,f,g,o in ref)
_OFF = [0, 512, 1536, 1024]
_NEG = -60000.0


def _split_sync_waits(nc, max_waits=1):
    """Walrus here rejects >1 sync-wait per instruction; push extras onto
    preceding same-engine NoOps."""
    import concourse.mybir as mybir

    ctr = 0
    for fn in nc.m.functions:
        for blk in fn.blocks:
            out = []
            changed = False
            for inst in blk.instructions:
                si = inst.sync_info
                if si is not None and si.on_wait and len(si.on_wait) > max_waits:
                    waits = list(si.on_wait)
                    extra, keep = waits[:-max_waits], waits[-max_waits:]
                    for i in range(0, len(extra), max_waits):
                        ctr += 1
                        nop = mybir.InstNoOp(
                            name=f"bass_waitsplit_{ctr}", ins=[], outs=[])
                        nop.engine = inst.engine
                        nop.sync_info = mybir.SyncInfo(
                            on_wait=extra[i:i + max_waits], on_update=[])
                        out.append(nop)
                    si.on_wait = keep
                    changed = True
                out.append(inst)
            if changed:
                blk.instructions[:] = out


def _gate_perm():
    """perm[tq*128+p] = original gate row for permuted position."""
    p = np.empty(4 * H, dtype=np.int64)
    for ttype in range(4):
        for qg in range(4):
            tq = ttype * 4 + qg
            base = _OFF[ttype] + qg * 128
            p[tq * 128:(tq + 1) * 128] = np.arange(base, base + 128)
    return p


def _build(nsteps=L, ntiles=NT):
    key = (nsteps, ntiles)
    if key in _BUILT:
        return _BUILT[key]
    import concourse.bass as bass
    import concourse.mybir as mybir
    import concourse.tile as tile

    f32 = mybir.dt.float32
    f16 = mybir.dt.float16
    SIG = mybir.ActivationFunctionType.Sigmoid
    TANH = mybir.ActivationFunctionType.Tanh
    MUL = mybir.AluOpType.mult
    SUB = mybir.AluOpType.subtract
    ngrp = nsteps // 4          # 4-step groups
    ntc = nsteps // 128 if nsteps >= 128 else 1   # epilogue t-chunks

    nc = bass.Bass()
    emb = nc.dram_tensor("emb", [V, E], f32, kind="ExternalInput")
    toks = nc.dram_tensor("toks", [128, ntiles], mybir.dt.int32,
                          kind="ExternalInput")
    tokmask = nc.dram_tensor("tokmask", [128, ntiles], f32,
                             kind="ExternalInput")
    wihT_d = nc.dram_tensor("wihT", [2, 4, 128, 2048], f16, kind="ExternalInput")
    whhT_d = nc.dram_tensor("whhT", [2, 4, 128, 2048], f16, kind="ExternalInput")
    biasW_d = nc.dram_tensor("biasW", [2, 16, 128], f16, kind="ExternalInput")
    biasInd_d = nc.dram_tensor("biasInd", [16, 512], f16, kind="ExternalInput")
    maskT_d = nc.dram_tensor("maskT", [2, 32 * ngrp], f16, kind="ExternalInput")
    identP_d = nc.dram_tensor("identP", [128, 128], f16, kind="ExternalInput")
    revP_d = nc.dram_tensor("revP", [128, 128], f16, kind="ExternalInput")
    out_d = nc.dram_tensor("out", [nsteps, NB, 2 * H], f32,
                           kind="ExternalOutput")

    with tile.TileContext(nc) as tc:
        with (
            tc.tile_pool(name="persist", bufs=1) as pp,
            tc.tile_pool(name="dram", bufs=1, space="DRAM") as dp,
            tc.tile_pool(name="p1", bufs=3) as p1,
            tc.tile_pool(name="tps", bufs=2, space="PSUM") as p1ps,
            tc.tile_pool(name="gps", bufs=3, space="PSUM") as gps,
            tc.tile_pool(name="sgp", bufs=2) as sgp,
            tc.tile_pool(name="ewp", bufs=2) as ewp,
            tc.tile_pool(name="cp", bufs=2) as cp,
            tc.tile_pool(name="fp", bufs=1) as fp,
        ):
            fps = p1ps
            # ---------------- persistent SBUF ----------------
            wihT = pp.tile([128, 2, 4, 2048], f16)
            whhT = pp.tile([128, 2, 4, 2048], f16)
            for d in range(2):
                for k in range(4):
                    nc.sync.dma_start(wihT[:, d, k, :], wihT_d[d, k])
                    nc.sync.dma_start(whhT[:, d, k, :], whhT_d[d, k])
            biasW = pp.tile([16, 2, 128], f16)
            nc.sync.dma_start(biasW[:, 0, :], biasW_d[0])
            nc.sync.dma_start(biasW[:, 1, :], biasW_d[1])
            biasInd = pp.tile([16, 512], f16)
            nc.sync.dma_start(biasInd[:], biasInd_d[:])
            maskT = pp.tile([1, 2, ngrp, 4, 8], f16)
            nc.sync.dma_start(maskT[:], maskT_d[:])
            ones1 = pp.tile([1, 128], f16)
            nc.vector.memset(ones1[:], 1.0)
            identP = pp.tile([128, 128], f16)
            nc.sync.dma_start(identP[:], identP_d[:])
            revP = pp.tile([128, 128], f16)
            nc.sync.dma_start(revP[:], revP_d[:])
            ones32 = pp.tile([128, 4, 8], f32)
            nc.vector.memset(ones32[:], 1.0)
            toks_t = pp.tile([128, ntiles], mybir.dt.int32)
            nc.sync.dma_start(toks_t[:], toks[:])
            tmask_t = pp.tile([128, ntiles], f32)
            nc.sync.dma_start(tmask_t[:], tokmask[:])

            xtf = pp.tile([128, 4, 4096], f16)
            xtb = pp.tile([128, 4, 4096], f16)
            hist = [pp.tile([128, 16, 4, 8], f16, name=f"hist{d}")
                    for d in range(2)]

            scr = dp.tile([2, 128, nsteps, 32], f16)

            # ---------------- phase 1: x -> xT (fwd + rev layouts) --------
            for r in range(ntiles):
                xt32 = p1.tile([128, 512], f32)
                nc.gpsimd.indirect_dma_start(
                    out=xt32[:], out_offset=None, in_=emb[:],
                    in_offset=bass.IndirectOffsetOnAxis(
                        ap=toks_t[:, r:r + 1], axis=0))
                xt16 = p1.tile([128, 512], f16)
                nc.vector.tensor_scalar_mul(xt16[:], xt32[:],
                                            tmask_t[:, r:r + 1])
                for k in range(4):
                    tp = p1ps.tile([128, 128], f16, space="PSUM")
                    nc.tensor.matmul(tp[:], xt16[:, 128 * k:128 * (k + 1)],
                                     identP[:], is_transpose=True)
                    eng = (r * 4 + k) % 2
                    dst = xtf[:, k, 128 * r:128 * (r + 1)]
                    if eng == 0:
                        nc.scalar.copy(dst, tp[:])
                    else:
                        nc.vector.tensor_copy(dst, tp[:])
                    tp2 = p1ps.tile([128, 128], f16, space="PSUM")
                    nc.tensor.matmul(tp2[:], xt16[:, 128 * k:128 * (k + 1)],
                                     revP[:], is_transpose=True)
                    rbase = 8 * ((ntiles - 1 - r) * 16)
                    dst2 = xtb[:, k, rbase:rbase + 128]
                    if eng == 0:
                        nc.vector.tensor_copy(dst2, tp2[:])
                    else:
                        nc.scalar.copy(dst2, tp2[:])

            # ---------------- phase 2: recurrence ----------------
            xts = [xtf, xtb]

            def prologue_open(g):
                """Open G banks for 4-step group g (both dirs): bias, mask."""
                tiles = []
                for d in range(2):
                    t = gps.tile([128, 4, 4, 4, 8], f32, space="PSUM",
                                 tag=f"G{d}")
                    nc.tensor.matmul(t[:], biasW[:, d, :], biasInd[:],
                                     start=True, stop=False,
                                     skip_group_check=True)
                    # padding mask: -60000 onto i/o gate rows at padded (t,b)
                    mrow = maskT[0:1, d, g]
                    mrow = mrow.unsqueeze(2).broadcast_to([1, 4, 4, 8])
                    for ttype in (0, 2):
                        nc.tensor.matmul(t[:, :, ttype, :, :], ones1[:],
                                         mrow, start=False, stop=False,
                                         skip_group_check=True)
                    tiles.append(t)
                return tiles

            def prologue_x(tiles, g, part):
                """x-part matmuls for 4 of the 16 gate chunks (both dirs)."""
                for d in range(2):
                    t = tiles[d]
                    for tq in range(4 * part, 4 * part + 4):
                        ttype, qg = tq >> 2, tq & 3
                        for k in range(4):
                            nc.tensor.matmul(
                                t[:, :, ttype, qg, :],
                                wihT[:, d, k, 128 * tq:128 * (tq + 1)],
                                xts[d][:, k, 32 * g:32 * (g + 1)],
                                start=False, stop=False,
                                skip_group_check=True)

            G = prologue_open(0)
            for part in range(4):
                prologue_x(G, 0, part)
            Gnext = None
            c_prev = [None, None]
            for s in range(nsteps):
                j, g = s % 4, s // 4
                # h-part matmuls
                if s > 0:
                    for d in range(2):
                        # hist is step-indexed for fwd, source-indexed for bwd
                        slot = (s - 1) % 16 if d == 0 else (nsteps - s) % 16
                        for tq in range(16):
                            ttype, qg = tq >> 2, tq & 3
                            for k in range(4):
                                last = (j == 3 and tq == 15 and k == 3)
                                nc.tensor.matmul(
                                    G[d][:, j, ttype, qg, :],
                                    whhT[:, d, k, 128 * tq:128 * (tq + 1)],
                                    hist[d][:, slot, k, :],
                                    start=False, stop=last,
                                    skip_group_check=True)
                elif nsteps >= 4:
                    pass  # group 0's stop rides on s=3's last h-matmul
                # activations + elementwise per dir
                for d in ((0, 1) if s % 2 == 0 else (1, 0)):
                    sg = sgp.tile([128, 4, 4, 8], f32, tag=f"sg{d}")
                    nc.scalar.activation(sg[:], G[d][:, j], SIG)
                    vg = ewp.tile([128, 4, 8], f32, tag=f"vg{d}")
                    nc.vector.scalar_tensor_tensor(
                        vg[:], sg[:, 3], 2.0, ones32[:], MUL, SUB)
                    cnew = cp.tile([128, 4, 8], f32, tag=f"c{d}")
                    if s == 0:
                        nc.vector.tensor_mul(cnew[:], sg[:, 0], vg[:])
                    else:
                        t2 = ewp.tile([128, 4, 8], f32, tag=f"t2{d}")
                        nc.gpsimd.tensor_mul(t2[:], sg[:, 1], c_prev[d])
                        t1 = ewp.tile([128, 4, 8], f32, tag=f"t1{d}")
                        nc.vector.tensor_mul(t1[:], sg[:, 0], vg[:])
                        nc.vector.tensor_add(cnew[:], t1[:], t2[:])
                    tch = ewp.tile([128, 4, 8], f32, tag=f"tch{d}")
                    nc.scalar.activation(tch[:], cnew[:], TANH)
                    wslot = s % 16 if d == 0 else (nsteps - 1 - s) % 16
                    nc.gpsimd.tensor_mul(hist[d][:, wslot], tch[:], sg[:, 2])
                    c_prev[d] = cnew
                    if s % 8 == 7:
                        lo = s - 7 if d == 0 else nsteps - 1 - s
                        slot0 = lo % 16
                        nc.sync.dma_start(
                            scr[d, :, lo:lo + 8, :],
                            hist[d][:, slot0:slot0 + 8])
                # next group's bias/mask plus a quarter of its x-part per step
                if g + 1 < ngrp:
                    if j == 0:
                        Gnext = prologue_open(g + 1)
                    if Gnext is not None:
                        prologue_x(Gnext, g + 1, j)
                if j == 3 and Gnext is not None:
                    G = Gnext
                    Gnext = None

            # ---------------- epilogue: scratch -> out ----------------
            tcn = max(1, nsteps // 128)
            tlen = min(nsteps, 128)
            for d in range(2):
                for tci in range(tcn):
                    ld = fp.tile([128, tlen, 4, 8], f16)
                    nc.sync.dma_start(
                        ld[:], scr[d, :, tlen * tci:tlen * (tci + 1), :])
                    asm = fp.tile([tlen, 8, 4, 128], f32)
                    for q in range(4):
                        for b in range(8):
                            tp = fps.tile([tlen, 128], f16, space="PSUM")
                            nc.tensor.matmul(tp[:], ld[:, :, q, b],
                                             identP[:], is_transpose=True)
                            if (q * 8 + b) % 2 == 0:
                                nc.scalar.copy(asm[:, b, q, :], tp[:])
                            else:
                                nc.vector.tensor_copy(asm[:, b, q, :], tp[:])
                    nc.sync.dma_start(
                        out_d[tlen * tci:tlen * (tci + 1), :,
                              512 * d:512 * (d + 1)], asm[:])

    _BUILT[key] = nc
    return nc


def _ensure_split(nc):
    if not getattr(nc, "_waitsplit_done", False):
        _split_sync_waits(nc)
        nc._waitsplit_done = True


def _host_const():
    perm = _gate_perm()
    scale = np.ones(4 * H, np.float32)
    scale[3 * 512:] = 2.0        # permuted g-block = last 512 rows
    biasInd = np.zeros((16, 512), np.float16)
    for k in range(16):
        ttype, qg = k >> 2, k & 3
        for s4 in range(4):
            base = s4 * 128 + ttype * 32 + qg * 8
            biasInd[k, base:base + 8] = 1.0
    identP = np.eye(128, dtype=np.float16)
    revP = np.zeros((128, 128), np.float16)
    for j in range(128):
        revP[(15 - j // 8) * 8 + (j % 8), j] = 1.0
    return perm, scale, biasInd, identP, revP


def _prep_weights(W_ih_f, W_hh_f, b_ih_f, b_hh_f, W_ih_b, W_hh_b, b_ih_b,
                  b_hh_b):
    perm, scale, biasInd, identP, revP = _host_const()

    def wt(W):
        Wp = np.asarray(W, np.float32)[perm] * scale[:, None]   # [2048, 512]
        return Wp.T.reshape(4, 128, 2048).astype(np.float16)

    wihT = np.stack([wt(W_ih_f), wt(W_ih_b)])
    whhT = np.stack([wt(W_hh_f), wt(W_hh_b)])
    bf = (np.asarray(b_ih_f, np.float32) + np.asarray(b_hh_f, np.float32))
    bb = (np.asarray(b_ih_b, np.float32) + np.asarray(b_hh_b, np.float32))
    bW = np.stack([(bf[perm] * scale).reshape(16, 128).astype(np.float16),
                   (bb[perm] * scale).reshape(16, 128).astype(np.float16)])
    return wihT, whhT, bW, biasInd.astype(np.float16), identP, revP


def _prep_core_inputs(c, tokens, mask, emb16, wihT, whhT, bW, biasInd,
                      identP, revP, nsteps, ntiles):
    s = slice(NB * c, NB * (c + 1))
    toks_c = np.clip(tokens[:nsteps, s], 0, V - 1).astype(np.int32)
    toks_c = toks_c.reshape(ntiles, 128).T
    tmask_c = mask[:nsteps, s].astype(np.float32).reshape(ntiles, 128).T
    mk = mask[:nsteps, s].astype(np.float32)          # [T, 8]
    maskT = np.stack([
        (_NEG * (1.0 - mk)).reshape(-1),
        (_NEG * (1.0 - mk[::-1])).reshape(-1),
    ]).astype(np.float16)                              # [2, T*8] (l-major)
    return {
        "emb": emb16,
        "toks": np.ascontiguousarray(toks_c),
        "tokmask": np.ascontiguousarray(tmask_c),
        "wihT": wihT,
        "whhT": whhT,
        "biasW": bW,
        "biasInd": biasInd,
        "maskT": maskT,
        "identP": identP,
        "revP": revP,
    }


def kernel(tokens, mask, emb_table, W_ih_f, W_hh_f, b_ih_f, b_hh_f,
           W_ih_b, W_hh_b, b_ih_b, b_hh_b, _nsteps=L, _cores=None,
           _trace=False):
    from concourse.bass_utils import run_bass_kernel_spmd

    tokens = np.asarray(tokens)
    mask = np.asarray(mask, dtype=np.float32)
    emb_f = np.ascontiguousarray(np.asarray(emb_table, dtype=np.float32))

    wihT, whhT, bW, biasInd, identP, revP = _prep_weights(
        W_ih_f, W_hh_f, b_ih_f, b_hh_f, W_ih_b, W_hh_b, b_ih_b, b_hh_b)

    nsteps = _nsteps
    ntiles = nsteps * NB // 128
    cores = list(range(NCORES)) if _cores is None else _cores

    nc = _build(nsteps, ntiles)
    _ensure_split(nc)
    in_maps = [
        _prep_core_inputs(c, tokens, mask, emb_f, wihT, whhT, bW, biasInd,
                          identP, revP, nsteps, ntiles)
        for c in cores
    ]
    res = run_bass_kernel_spmd(nc, in_maps, core_ids=cores, trace=_trace)
    out = np.empty((nsteps, len(cores) * NB, 2 * H), np.float32)
    for i, c in enumerate(cores):
        out[:, NB * i:NB * (i + 1), :] = res.results[i]["out"]
    kernel._last_results = res
    return out


# revision 38
# speedup vs baseline: 28.3567x; 1.0185x over previous
"""Bidirectional LSTM (L=512, B=64, E=512, H=512 per dir) on 8 NeuronCores.

Strategy (SPMD, zero cross-core communication, batch-parallel: core c owns
samples [8c, 8c+8), both directions):

Transposed recurrence: gates live on PSUM *partitions* (128 gate rows per
(ttype, qg) chunk), batch+hidden-chunk on the free dim.  Cost model charges
matmuls by output free size only, so out free = 8 (batch) makes each of the
64 h-matmuls per direction-step ~3 ns instead of streaming 512x4-cycle fp32
columns like the row-major formulation.

  - All matmul operands fp16 (PSUM accumulates fp32 exactly).
  - tanh folded to sigmoid: tanh(x) = 2*sigmoid(2x)-1, with the 2x folded
    into the g-gate rows of Wih/Whh/bias, so one sigmoid covers all 128
    gate columns per step; the affine 2u-1 runs on DVE/GPSIMD.
  - Gate layout per step column block: c = ttype*32 + b*4 + q with ttype
    order (i, f, o, g); hidden index = q*128 + p.  G PSUM tile [128,4,4,8,4]
    = one full 2KB bank = 4 steps; opened by a K=16 bias matmul
    (start=True zeroes the bank), then mask (K=1), x-part (free 32, batched
    over 4 steps), then per-step h-matmuls; stop on the last h-matmul.
  - Padding: x rows zeroed via embedding mask; i/o gate pre-activations get
    -60000 at padded (t,b) via a rank-1 matmul from a per-4-step-group mask
    table, so sigmoid(i)=sigmoid(o)=0 => h=0 and c propagates exactly like
    the reference's h*m, c*m masking (mask is monotone per direction).
  - fwd and bwd are independent recurrences interleaved step-by-step: two
    chains hide each other's serial activation->elementwise latency.
  - h written per step in fp16 to a history tile; every 8 steps one DMA
    flushes to a DRAM scratch laid out [d, p, t, (b,q)].  The epilogue
    PE-transposes the scratch into [t, b, hidden] fp32 output (fp16
    precision is ample for rel tol 2e-2).
"""

import sys

sys.path.insert(0, "/opt/trn_rl_repo")

import numpy as np

L, B, E, V = 512, 64, 512, 32000
H = 512
NB = 8
NCORES = 8
NT = L * NB // 128      # 32 gather tiles of 128 (l,b) rows

_BUILT = {}

# ttype order (i, f, o, g) -> original row block offsets (i,f,g,o in ref)
_OFF = [0, 512, 1536, 1024]
_NEG = -60000.0


def _split_sync_waits(nc, max_waits=1):
    """Walrus here rejects >1 sync-wait per instruction; push extras onto
    preceding same-engine NoOps."""
    import concourse.mybir as mybir

    ctr = 0
    for fn in nc.m.functions:
        for blk in fn.blocks:
            out = []
            changed = False
            for inst in blk.instructions:
                si = inst.sync_info
                if si is not None and si.on_wait and len(si.on_wait) > max_waits:
                    waits = list(si.on_wait)
                    extra, keep = waits[:-max_waits], waits[-max_waits:]
                    for i in range(0, len(extra), max_waits):
                        ctr += 1
                        nop = mybir.InstNoOp(
                            name=f"bass_waitsplit_{ctr}", ins=[], outs=[])
                        nop.engine = inst.engine
                        nop.sync_info = mybir.SyncInfo(
                            on_wait=extra[i:i + max_waits], on_update=[])
                        out.append(nop)
                    si.on_wait = keep
                    changed = True
                out.append(inst)
            if changed:
                blk.instructions[:] = out


def _gate_perm():
    """perm[tq*128+p] = original gate row for permuted position."""
    p = np.empty(4 * H, dtype=np.int64)
    for ttype in range(4):
        for qg in range(4):
            tq = ttype * 4 + qg
            base = _OFF[ttype] + qg * 128
            p[tq * 128:(tq + 1) * 128] = np.arange(base, base + 128)
    return p


def _build(nsteps=L, ntiles=NT):
    key = (nsteps, ntiles)
    if key in _BUILT:
        return _BUILT[key]
    import concourse.bass as bass
    import concourse.mybir as mybir
    import concourse.tile as tile

    f32 = mybir.dt.float32
    f16 = mybir.dt.float16
    SIG = mybir.ActivationFunctionType.Sigmoid
    TANH = mybir.ActivationFunctionType.Tanh
    MUL = mybir.AluOpType.mult
    SUB = mybir.AluOpType.subtract
    ngrp = nsteps // 4          # 4-step groups
    ntc = nsteps // 128 if nsteps >= 128 else 1   # epilogue t-chunks

    nc = bass.Bass()
    emb = nc.dram_tensor("emb", [V, E], f32, kind="ExternalInput")
    toks = nc.dram_tensor("toks", [128, ntiles], mybir.dt.int32,
                          kind="ExternalInput")
    tokmask = nc.dram_tensor("tokmask", [128, ntiles], f32,
                             kind="ExternalInput")
    wihT_d = nc.dram_tensor("wihT", [2, 4, 128, 2048], f16, kind="ExternalInput")
    whhT_d = nc.dram_tensor("whhT", [2, 4, 128, 2048], f16, kind="ExternalInput")
    biasW_d = nc.dram_tensor("biasW", [2, 16, 128], f16, kind="ExternalInput")
    biasInd_d = nc.dram_tensor("biasInd", [16, 512], f16, kind="ExternalInput")
    maskT_d = nc.dram_tensor("maskT", [2, 32 * ngrp], f16, kind="ExternalInput")
    identP_d = nc.dram_tensor("identP", [128, 128], f16, kind="ExternalInput")
    revP_d = nc.dram_tensor("revP", [128, 128], f16, kind="ExternalInput")
    out_d = nc.dram_tensor("out", [nsteps, NB, 2 * H], f32,
                           kind="ExternalOutput")

    with tile.TileContext(nc) as tc:
        with (
            tc.tile_pool(name="persist", bufs=1) as pp,
            tc.tile_pool(name="dram", bufs=1, space="DRAM") as dp,
            tc.tile_pool(name="p1", bufs=3) as p1,
            tc.tile_pool(name="tps", bufs=2, space="PSUM") as p1ps,
            tc.tile_pool(name="gps", bufs=2, space="PSUM") as gps,
            tc.tile_pool(name="sgp", bufs=2) as sgp,
            tc.tile_pool(name="ewp", bufs=2) as ewp,
            tc.tile_pool(name="cp", bufs=2) as cp,
            tc.tile_pool(name="fp", bufs=1) as fp,
        ):
            fps = p1ps
            # ---------------- persistent SBUF ----------------
            wihT = pp.tile([128, 2, 4, 2048], f16)
            whhT = pp.tile([128, 2, 4, 2048], f16)
            for d in range(2):
                for k in range(4):
                    nc.sync.dma_start(wihT[:, d, k, :], wihT_d[d, k])
                    nc.sync.dma_start(whhT[:, d, k, :], whhT_d[d, k])
            biasW = pp.tile([16, 2, 128], f16)
            nc.sync.dma_start(biasW[:, 0, :], biasW_d[0])
            nc.sync.dma_start(biasW[:, 1, :], biasW_d[1])
            biasInd = pp.tile([16, 512], f16)
            nc.sync.dma_start(biasInd[:], biasInd_d[:])
            maskT = pp.tile([1, 2, ngrp, 4, 8], f16)
            nc.sync.dma_start(maskT[:], maskT_d[:])
            ones1 = pp.tile([1, 128], f16)
            nc.vector.memset(ones1[:], 1.0)
            identP = pp.tile([128, 128], f16)
            nc.sync.dma_start(identP[:], identP_d[:])
            revP = pp.tile([128, 128], f16)
            nc.sync.dma_start(revP[:], revP_d[:])
            ones32 = pp.tile([128, 4, 8], f32)
            nc.vector.memset(ones32[:], 1.0)
            toks_t = pp.tile([128, ntiles], mybir.dt.int32)
            nc.sync.dma_start(toks_t[:], toks[:])
            tmask_t = pp.tile([128, ntiles], f32)
            nc.sync.dma_start(tmask_t[:], tokmask[:])

            xtf = pp.tile([128, 4, 4096], f16)
            xtb = pp.tile([128, 4, 4096], f16)
            hist = [pp.tile([128, 16, 4, 8], f16, name=f"hist{d}")
                    for d in range(2)]

            scr = dp.tile([2, 128, nsteps, 32], f16)

            # ---------------- phase 1: x -> xT (fwd + rev layouts) --------
            for r in range(ntiles):
                xt32 = p1.tile([128, 512], f32)
                nc.gpsimd.indirect_dma_start(
                    out=xt32[:], out_offset=None, in_=emb[:],
                    in_offset=bass.IndirectOffsetOnAxis(
                        ap=toks_t[:, r:r + 1], axis=0))
                xt16 = p1.tile([128, 512], f16)
                nc.vector.tensor_scalar_mul(xt16[:], xt32[:],
                                            tmask_t[:, r:r + 1])
                for k in range(4):
                    tp = p1ps.tile([128, 128], f16, space="PSUM")
                    nc.tensor.matmul(tp[:], xt16[:, 128 * k:128 * (k + 1)],
                                     identP[:], is_transpose=True)
                    eng = (r * 4 + k) % 2
                    dst = xtf[:, k, 128 * r:128 * (r + 1)]
                    if eng == 0:
                        nc.scalar.copy(dst, tp[:])
                    else:
                        nc.vector.tensor_copy(dst, tp[:])
                    tp2 = p1ps.tile([128, 128], f16, space="PSUM")
                    nc.tensor.matmul(tp2[:], xt16[:, 128 * k:128 * (k + 1)],
                                     revP[:], is_transpose=True)
                    rbase = 8 * ((ntiles - 1 - r) * 16)
                    dst2 = xtb[:, k, rbase:rbase + 128]
                    if eng == 0:
                        nc.vector.tensor_copy(dst2, tp2[:])
                    else:
                        nc.scalar.copy(dst2, tp2[:])

            # ---------------- phase 2: recurrence ----------------
            xts = [xtf, xtb]

            def prologue_open(g):
                """Open G banks for 4-step group g (both dirs): bias, mask."""
                tiles = []
                for d in range(2):
                    t = gps.tile([128, 4, 4, 4, 8], f32, space="PSUM",
                                 tag=f"G{d}")
                    nc.tensor.matmul(t[:], biasW[:, d, :], biasInd[:],
                                     start=True, stop=False,
                                     skip_group_check=True)
                    # padding mask: -60000 onto i/o gate rows at padded (t,b)
                    mrow = maskT[0:1, d, g]
                    mrow = mrow.unsqueeze(2).broadcast_to([1, 4, 4, 8])
                    for ttype in (0, 2):
                        nc.tensor.matmul(t[:, :, ttype, :, :], ones1[:],
                                         mrow, start=False, stop=False,
                                         skip_group_check=True)
                    tiles.append(t)
                return tiles

            def prologue_x(tiles, g, part):
                """x-part matmuls for 4 of the 16 gate chunks (both dirs)."""
                for d in range(2):
                    t = tiles[d]
                    for tq in range(4 * part, 4 * part + 4):
                        ttype, qg = tq >> 2, tq & 3
                        for k in range(4):
                            nc.tensor.matmul(
                                t[:, :, ttype, qg, :],
                                wihT[:, d, k, 128 * tq:128 * (tq + 1)],
                                xts[d][:, k, 32 * g:32 * (g + 1)],
                                start=False, stop=False,
                                skip_group_check=True)

            G = prologue_open(0)
            for part in range(4):
                prologue_x(G, 0, part)
            Gnext = None
            c_prev = [None, None]
            for s in range(nsteps):
                j, g = s % 4, s // 4
                # h-part matmuls
                if s > 0:
                    for d in range(2):
                        # hist is step-indexed for fwd, source-indexed for bwd
                        slot = (s - 1) % 16 if d == 0 else (nsteps - s) % 16
                        for tq in range(16):
                            ttype, qg = tq >> 2, tq & 3
                            for k in range(4):
                                last = (j == 3 and tq == 15 and k == 3)
                                nc.tensor.matmul(
                                    G[d][:, j, ttype, qg, :],
                                    whhT[:, d, k, 128 * tq:128 * (tq + 1)],
                                    hist[d][:, slot, k, :],
                                    start=False, stop=last,
                                    skip_group_check=True)
                elif nsteps >= 4:
                    pass  # group 0's stop rides on s=3's last h-matmul
                # activations + elementwise per dir
                for d in ((0, 1) if s % 2 == 0 else (1, 0)):
                    sg = sgp.tile([128, 4, 4, 8], f32, tag=f"sg{d}")
                    nc.scalar.activation(sg[:], G[d][:, j], SIG)
                    vg = ewp.tile([128, 4, 8], f32, tag=f"vg{d}")
                    nc.vector.scalar_tensor_tensor(
                        vg[:], sg[:, 3], 2.0, ones32[:], MUL, SUB)
                    cnew = cp.tile([128, 4, 8], f32, tag=f"c{d}")
                    if s == 0:
                        nc.vector.tensor_mul(cnew[:], sg[:, 0], vg[:])
                    else:
                        t2 = ewp.tile([128, 4, 8], f32, tag=f"t2{d}")
                        nc.gpsimd.tensor_mul(t2[:], sg[:, 1], c_prev[d])
                        t1 = ewp.tile([128, 4, 8], f32, tag=f"t1{d}")
                        nc.vector.tensor_mul(t1[:], sg[:, 0], vg[:])
                        nc.vector.tensor_add(cnew[:], t1[:], t2[:])
                    tch = ewp.tile([128, 4, 8], f32, tag=f"tch{d}")
                    nc.scalar.activation(tch[:], cnew[:], TANH)
                    wslot = s % 16 if d == 0 else (nsteps - 1 - s) % 16
                    nc.gpsimd.tensor_mul(hist[d][:, wslot], tch[:], sg[:, 2])
                    c_prev[d] = cnew
                    if s % 8 == 7:
                        lo = s - 7 if d == 0 else nsteps - 1 - s
                        slot0 = lo % 16
                        nc.sync.dma_start(
                            scr[d, :, lo:lo + 8, :],
                            hist[d][:, slot0:slot0 + 8])
                # next group's bias/mask plus a quarter of its x-part per step
                if g + 1 < ngrp:
                    if j == 0:
                        Gnext = prologue_open(g + 1)
                    if Gnext is not None:
                        prologue_x(Gnext, g + 1, j)
                if j == 3 and Gnext is not None:
                    G = Gnext
                    Gnext = None

            # ---------------- epilogue: scratch -> out ----------------
            tcn = max(1, nsteps // 128)
            tlen = min(nsteps, 128)
            for d in range(2):
                for tci in range(tcn):
                    ld = fp.tile([128, tlen, 4, 8], f16)
                    nc.sync.dma_start(
                        ld[:], scr[d, :, tlen * tci:tlen * (tci + 1), :])
                    asm = fp.tile([tlen, 8, 4, 128], f32)
                    for q in range(4):
                        for b in range(8):
                            tp = fps.tile([tlen, 128], f16, space="PSUM")
                            nc.tensor.matmul(tp[:], ld[:, :, q, b],
                                             identP[:], is_transpose=True)
                            if (q * 8 + b) % 2 == 0:
                                nc.scalar.copy(asm[:, b, q, :], tp[:])
                            else:
                                nc.vector.tensor_copy(asm[:, b, q, :], tp[:])
                    nc.sync.dma_start(
                        out_d[tlen * tci:tlen * (tci + 1), :,
                              512 * d:512 * (d + 1)], asm[:])

    _BUILT[key] = nc
    return nc


def _ensure_split(nc):
    if not getattr(nc, "_waitsplit_done", False):
        _split_sync_waits(nc)
        nc._waitsplit_done = True


def _host_const():
    perm = _gate_perm()
    scale = np.ones(4 * H, np.float32)
    scale[3 * 512:] = 2.0        # permuted g-block = last 512 rows
    biasInd = np.zeros((16, 512), np.float16)
    for k in range(16):
        ttype, qg = k >> 2, k & 3
        for s4 in range(4):
            base = s4 * 128 + ttype * 32 + qg * 8
            biasInd[k, base:base + 8] = 1.0
    identP = np.eye(128, dtype=np.float16)
    revP = np.zeros((128, 128), np.float16)
    for j in range(128):
        revP[(15 - j // 8) * 8 + (j % 8), j] = 1.0
    return perm, scale, biasInd, identP, revP


def _prep_weights(W_ih_f, W_hh_f, b_ih_f, b_hh_f, W_ih_b, W_hh_b, b_ih_b,
                  b_hh_b):
    perm, scale, biasInd, identP, revP = _host_const()

    def wt(W):
        Wp = np.asarray(W, np.float32)[perm] * scale[:, None]   # [2048, 512]
        return Wp.T.reshape(4, 128, 2048).astype(np.float16)

    wihT = np.stack([wt(W_ih_f), wt(W_ih_b)])
    whhT = np.stack([wt(W_hh_f), wt(W_hh_b)])
    bf = (np.asarray(b_ih_f, np.float32) + np.asarray(b_hh_f, np.float32))
    bb = (np.asarray(b_ih_b, np.float32) + np.asarray(b_hh_b, np.float32))
    bW = np.stack([(bf[perm] * scale).reshape(16, 128).astype(np.float16),
                   (bb[perm] * scale).reshape(16, 128).astype(np.float16)])
    return wihT, whhT, bW, biasInd.astype(np.float16), identP, revP


def _prep_core_inputs(c, tokens, mask, emb16, wihT, whhT, bW, biasInd,
                      identP, revP, nsteps, ntiles):
    s = slice(NB * c, NB * (c + 1))
    toks_c = np.clip(tokens[:nsteps, s], 0, V - 1).astype(np.int32)
    toks_c = toks_c.reshape(ntiles, 128).T
    tmask_c = mask[:nsteps, s].astype(np.float32).reshape(ntiles, 128).T
    mk = mask[:nsteps, s].astype(np.float32)          # [T, 8]
    maskT = np.stack([
        (_NEG * (1.0 - mk)).reshape(-1),
        (_NEG * (1.0 - mk[::-1])).reshape(-1),
    ]).astype(np.float16)                              # [2, T*8] (l-major)
    return {
        "emb": emb16,
        "toks": np.ascontiguousarray(toks_c),
        "tokmask": np.ascontiguousarray(tmask_c),
        "wihT": wihT,
        "whhT": whhT,
        "biasW": bW,
        "biasInd": biasInd,
        "maskT": maskT,
        "identP": identP,
        "revP": revP,
    }


def kernel(tokens, mask, emb_table, W_ih_f, W_hh_f, b_ih_f, b_hh_f,
           W_ih_b, W_hh_b, b_ih_b, b_hh_b, _nsteps=L, _cores=None,
           _trace=False):
    from concourse.bass_utils import run_bass_kernel_spmd

    tokens = np.asarray(tokens)
    mask = np.asarray(mask, dtype=np.float32)
    emb_f = np.ascontiguousarray(np.asarray(emb_table, dtype=np.float32))

    wihT, whhT, bW, biasInd, identP, revP = _prep_weights(
        W_ih_f, W_hh_f, b_ih_f, b_hh_f, W_ih_b, W_hh_b, b_ih_b, b_hh_b)

    nsteps = _nsteps
    ntiles = nsteps * NB // 128
    cores = list(range(NCORES)) if _cores is None else _cores

    nc = _build(nsteps, ntiles)
    _ensure_split(nc)
    in_maps = [
        _prep_core_inputs(c, tokens, mask, emb_f, wihT, whhT, bW, biasInd,
                          identP, revP, nsteps, ntiles)
        for c in cores
    ]
    res = run_bass_kernel_spmd(nc, in_maps, core_ids=cores, trace=_trace)
    out = np.empty((nsteps, len(cores) * NB, 2 * H), np.float32)
    for i, c in enumerate(cores):
        out[:, NB * i:NB * (i + 1), :] = res.results[i]["out"]
    kernel._last_results = res
    return out
